# revision 4
# baseline (speedup 1.0000x reference)
"""BoundaryEnhancedLoss on 8 TRN2 NeuronCores — data-parallel over batch.

Math (2-class specialization of the reference):
  d = pred[:,1] - pred[:,0]
  ce_pix = softplus(d) - t*d = -ln(pt)  where pt = sigmoid((2t-1)*d)
  focal_pix = 0.25*(1-pt)^2*ce_pix
  prob1 = sigmoid(d);  with t in {0,1}:  t*prob1 = t*pt  and
  prob1 = 1 - t - pt + 2*t*pt
  boundary bnd = [0 < s < 25], s = 5x5 box-sum of t (zero pad)
  Per-image sums needed (bnd in {0,1}):
    S1 = sum bnd, S2 = sum t*bnd, S3 = sum pt*bnd, S4 = sum pt*t*bnd
    inter = S4,  union = S1 - S3 + 2*S4
  Global: L = sum ln(pt)  (ce_sum = -L),  F = sum (pt-1)^2*ln(pt)
    (focal_sum = -F)

Each core handles 4 images; device emits per-partition partial sums
(stats[128, 72]); host reduces and combines.
"""
import numpy as np
import ml_dtypes
from contextlib import ExitStack

import concourse.bass as bass
import concourse.tile as tile
from concourse import bacc, mybir
from concourse.bass_utils import run_bass_kernel_spmd

BF16 = mybir.dt.bfloat16
F32 = mybir.dt.float32
Alu = mybir.AluOpType
Act = mybir.ActivationFunctionType

NCORES = 8
BPC = 4          # images per core
H = W = 512
P = 128
NCHUNK = H // P  # 4 row-chunks per image
NPIX = 32 * H * W
NST = 18         # stat columns per chunk
STW = NCHUNK * NST  # 72


def _band_consts():
    bmain = np.zeros((P, P), dtype=np.float32)
    for k in range(P):
        bmain[k, max(0, k - 2):min(P, k + 3)] = 1.0
    # halo bands embedded at the real source partitions (full-height lhsT;
    # matmul requires base partition 0)
    btop = np.zeros((P, P), dtype=np.float32)
    btop[126, 0] = 1.0
    btop[127, 0] = btop[127, 1] = 1.0
    bbot = np.zeros((P, P), dtype=np.float32)
    bbot[0, 126] = bbot[0, 127] = 1.0
    bbot[1, 127] = 1.0
    bf = ml_dtypes.bfloat16
    return bmain.astype(bf), btop.astype(bf), bbot.astype(bf)


def build_nc():
    nc = bacc.Bacc("TRN2", target_bir_lowering=False, debug=False,
                   num_devices=NCORES)
    pred = nc.dram_tensor("pred", [BPC, 2, H, W], F32, kind="ExternalInput")
    tgt = nc.dram_tensor("tgt", [BPC, H, W], BF16, kind="ExternalInput")
    bmain = nc.dram_tensor("bmain", [P, P], BF16, kind="ExternalInput")
    btop = nc.dram_tensor("btop", [P, P], BF16, kind="ExternalInput")
    bbot = nc.dram_tensor("bbot", [P, P], BF16, kind="ExternalInput")
    stats = nc.dram_tensor("stats", [P, STW], F32, kind="ExternalOutput")

    with tile.TileContext(nc) as tc, ExitStack() as ctx:
        persist = ctx.enter_context(tc.tile_pool(name="persist", bufs=1))
        work = ctx.enter_context(tc.tile_pool(name="work", bufs=2))
        psum = ctx.enter_context(tc.tile_pool(name="psum", bufs=2, space="PSUM"))

        bias24 = persist.tile([P, 1], F32, tag="bias24")
        nc.gpsimd.memset(bias24[:], -24.0)
        bmain_t = persist.tile([P, P], BF16, tag="bmain")
        btop_t = persist.tile([P, P], BF16, tag="btop")
        bbot_t = persist.tile([P, P], BF16, tag="bbot")
        nc.sync.dma_start(bmain_t[:], bmain[:])
        nc.sync.dma_start(btop_t[:], btop[:])
        nc.sync.dma_start(bbot_t[:], bbot[:])

        t_tiles, c_tiles, pt_tiles, st_tiles = [], [], [], []
        for r in range(NCHUNK):
            t_tiles.append(persist.tile([P, BPC, W + 4], BF16, tag=f"t{r}", name=f"t{r}"))
            c_tiles.append(persist.tile([P, BPC, W], BF16, tag=f"c{r}", name=f"c{r}"))
            pt_tiles.append(persist.tile([P, BPC, W], BF16, tag=f"pt{r}", name=f"pt{r}"))
            st_tiles.append(persist.tile([P, NST], F32, tag=f"st{r}", name=f"st{r}"))

        # ---- Loop 1: load t, W-direction 5-tap box sum (c) ----
        for r in range(NCHUNK):
            tr, cr = t_tiles[r], c_tiles[r]
            nc.gpsimd.memset(tr[:, :, 0:2], 0.0)
            nc.gpsimd.memset(tr[:, :, W + 2:W + 4], 0.0)
            src = tgt[:, bass.ts(r, P), :].rearrange("b p w -> p b w")
            nc.sync.dma_start(tr[:, :, 2:W + 2], src)
            a = work.tile([P, BPC, W + 3], BF16, tag="wca")
            nc.gpsimd.tensor_tensor(a[:], tr[:, :, 0:W + 3], tr[:, :, 1:W + 4],
                                    op=Alu.add)
            b2 = work.tile([P, BPC, W], BF16, tag="wcb")
            nc.gpsimd.tensor_tensor(b2[:], a[:, :, 0:W], a[:, :, 2:W + 2],
                                    op=Alu.add)
            nc.vector.tensor_tensor(cr[:], b2[:], tr[:, :, 4:W + 4], op=Alu.add)

        # ---- Loop 2: pred chunks, pt, boundary, dice partial sums ----
        for r in range(NCHUNK):
            tr, cr, ptr, st = t_tiles[r], c_tiles[r], pt_tiles[r], st_tiles[r]
            p0 = work.tile([P, BPC, W], F32, tag="p0")
            p1 = work.tile([P, BPC, W], F32, tag="p1")
            nc.sync.dma_start(
                p0[:], pred[:, 0, bass.ts(r, P), :].rearrange("b p w -> p b w"))
            nc.sync.dma_start(
                p1[:], pred[:, 1, bass.ts(r, P), :].rearrange("b p w -> p b w"))
            d = work.tile([P, BPC, W], BF16, tag="d")
            nc.vector.tensor_tensor(d[:], p1[:], p0[:], op=Alu.subtract)
            hs = work.tile([P, BPC, W], BF16, tag="hs")
            nc.vector.scalar_tensor_tensor(
                hs[:], tr[:, :, 2:W + 2], 0.5, d[:],
                op0=Alu.subtract, op1=Alu.mult)
            # pt = sigmoid(2*(t-0.5)*d)
            nc.scalar.activation(ptr[:], hs[:], Act.Sigmoid, scale=2.0)

            # H-direction band matmuls into PSUM (per image = per bank)
            s = psum.tile([P, BPC, W], F32, tag="s")
            for i in range(BPC):
                last = (r == NCHUNK - 1)
                nc.tensor.matmul(s[:, i, :], bmain_t[:], cr[:, i, :],
                                 start=True, stop=(r == 0 and False) or False)
                if r > 0:
                    nc.tensor.matmul(s[:, i, :], btop_t[:],
                                     c_tiles[r - 1][:, i, :],
                                     start=False, stop=last)
                if r < NCHUNK - 1:
                    nc.tensor.matmul(s[:, i, :], bbot_t[:],
                                     c_tiles[r + 1][:, i, :],
                                     start=False, stop=True)

            ero = work.tile([P, BPC, W], BF16, tag="ero")
            nc.scalar.activation(ero[:], s[:], Act.Relu, bias=bias24[:])
            bnd = work.tile([P, BPC, W], BF16, tag="bnd")
            tbnd = work.tile([P, BPC, W], BF16, tag="tbnd")
            s3o = work.tile([P, BPC, W], BF16, tag="s3o")
            s4o = work.tile([P, BPC, W], BF16, tag="s4o")
            for i in range(BPC):
                nc.vector.scalar_tensor_tensor(
                    bnd[:, i, :], s[:, i, :], 0.5, ero[:, i, :],
                    op0=Alu.is_ge, op1=Alu.subtract,
                    accum_out=st[:, 0 + i:1 + i])
                nc.vector.scalar_tensor_tensor(
                    tbnd[:, i, :], tr[:, i, 2:W + 2], 1.0, bnd[:, i, :],
                    op0=Alu.mult, op1=Alu.mult,
                    accum_out=st[:, 4 + i:5 + i])
                nc.vector.scalar_tensor_tensor(
                    s3o[:, i, :], ptr[:, i, :], 1.0, bnd[:, i, :],
                    op0=Alu.mult, op1=Alu.mult,
                    accum_out=st[:, 8 + i:9 + i])
                nc.vector.scalar_tensor_tensor(
                    s4o[:, i, :], ptr[:, i, :], 1.0, tbnd[:, i, :],
                    op0=Alu.mult, op1=Alu.mult,
                    accum_out=st[:, 12 + i:13 + i])

        # ---- Loop 3: ln(pt), focal ----
        for r in range(NCHUNK):
            ptr, st = pt_tiles[r], st_tiles[r]
            lnp = work.tile([P, BPC, W], BF16, tag="lnp")
            nc.scalar.activation(lnp[:], ptr[:], Act.Ln,
                                 accum_out=st[:, 16:17])
            q2 = work.tile([P, BPC, W], BF16, tag="q2")
            nc.vector.scalar_tensor_tensor(
                q2[:], ptr[:], 1.0, lnp[:], op0=Alu.subtract, op1=Alu.mult)
            fo = work.tile([P, BPC, W], BF16, tag="fo")
            nc.vector.scalar_tensor_tensor(
                fo[:], ptr[:], 1.0, q2[:], op0=Alu.subtract, op1=Alu.mult,
                accum_out=st[:, 17:18])

        for r in range(NCHUNK):
            nc.sync.dma_start(stats[:, bass.ts(r, NST)], st_tiles[r][:])

    nc.compile()
    return nc


_NC = None


def _get_nc():
    global _NC
    if _NC is None:
        _NC = build_nc()
    return _NC


def _host_combine(stats_all):
    """stats_all: list of 8 arrays [128, 72] f32 -> final loss (np.float32)."""
    S1 = np.zeros(32, np.float64)
    S2 = np.zeros(32, np.float64)
    S3 = np.zeros(32, np.float64)
    S4 = np.zeros(32, np.float64)
    L = 0.0
    F = 0.0
    for core, stm in enumerate(stats_all):
        st = stm.astype(np.float64).sum(axis=0)  # [72] summed over partitions
        for r in range(NCHUNK):
            base = r * NST
            for i in range(BPC):
                g = core * BPC + i
                S1[g] += st[base + 0 + i]
                S2[g] += st[base + 4 + i]
                S3[g] += st[base + 8 + i]
                S4[g] += st[base + 12 + i]
            L += st[base + 16]
            F += st[base + 17]
    ce_loss = (-L) / NPIX
    focal = 0.25 * (-F) / NPIX
    inter = S4
    union = S1 - S3 + 2.0 * S4
    dice = 2.0 * inter / (union + 1e-8)
    bdice = 1.0 - dice.mean()
    return np.float32(ce_loss + focal + bdice)


def run_cores(pred, target, trace=False):
    """Shard, run on 8 cores, return (stats_all, exec_time_ns)."""
    nc = _get_nc()
    bmain, btop, bbot = _band_consts()
    tgt_bf = target.astype(np.float32).astype(ml_dtypes.bfloat16)
    pred = np.asarray(pred, dtype=np.float32)
    in_maps = []
    for core in range(NCORES):
        sl = slice(core * BPC, (core + 1) * BPC)
        in_maps.append({
            "pred": pred[sl],
            "tgt": tgt_bf[sl],
            "bmain": bmain,
            "btop": btop,
            "bbot": bbot,
        })
    res = run_bass_kernel_spmd(nc, in_maps, list(range(NCORES)), trace=trace)
    stats_all = [res.results[c]["stats"] for c in range(NCORES)]
    return stats_all, res.exec_time_ns


def kernel(pred, target):
    stats_all, _ = run_cores(pred, target, trace=False)
    return _host_combine(stats_all)
